# revision 4
# baseline (speedup 1.0000x reference)
"""Trainium2 Bass kernel for nn_FAPELoss (B=2, R=1024, A=4096) on 8 NeuronCores.

Decomposition (all heavy compute on device):
  FAPE:  err^2[b,r,a] = <msym[b,r], q[b,a]> (28-dim symmetric-packed quadratic
         form of x7 = [coords_pred, coords_true, 1]) -> K=28 fp32r matmuls,
         [128 frames x 2048 atoms] PSUM tiles (4 banks).  Frames sharded
         across cores.  Per tile: ACT sqrt(err^2 + BIAS) from PSUM (BIAS makes
         the argument provably positive under fp32r rounding; systematic error
         ~BIAS/(2*err) per point, far below tolerance), then DVE
         min(.,10)+row-accumulate in bf16 2x mode.
  Clash: u = d^2 - (r_i+r_j)^2 straight out of K=6 fp32r matmuls
         (weights [-2x; |x|^2-r^2; 1; -2r], moving [y; 1; |y|^2-r^2; r]);
         clash pair <=> u < 0 (the reference's d>EPS arm is always true due to
         its 1e-12 floor).  Upper-block-triangle of the symmetric AxA matrix
         only: count = S_upper + S_diagblocks/2.  Four [128x512] blocks of the
         same kind share one [128x2048] PSUM tile; the count is a single DVE
         tensor_scalar(is_lt,accum) or ACT Sign(accum) per quad tile, split
         across both engines for balance.
  Physics: C/N atoms compacted on host (~220 of 4096 each) into a padded
         [384x384] problem; penalty relu(|d-1.33|-0.2), masked, accumulated.
         One [128x384] tile per core (6 used, 2 dummy).
Final tiny reductions (128-partition sums, res_mask weighting, denominators)
happen on host from a [128, 32] per-core accumulator tensor.
"""
import numpy as np

import concourse.bacc as bacc
import concourse.mybir as mybir
from concourse.tile import TileContext
from concourse.bass_utils import run_bass_kernel_spmd

F32 = mybir.dt.float32
F32R = mybir.dt.float32r
BF16 = mybir.dt.bfloat16
ALU = mybir.AluOpType
ACTF = mybir.ActivationFunctionType

# Problem constants (fixed by the module being modelled).
B, R, A = 2, 1024, 4096
NCORES = 8
RS = R // NCORES               # frames per core per batch = 128
CLAMP_DIST = 10.0
EPS = 1e-8
SQRT_BIAS = 0.02               # positivity guard for sqrt under fp32r rounding
C_IDX, N_IDX = 0, 1
CLASH_W, PHYS_W = 0.05, 0.3

# FAPE quad tiles: [128 frames x 2048 atoms]; 2 per batch -> 4 per core.
FAPE_TILES = [(b, h) for b in range(B) for h in range(A // 2048)]  # 4

# Clash quad tiles: the AxA (per batch) matrix in [128 x 512] blocks, upper
# block-triangle; four same-kind blocks share one [128 x 2048] PSUM tile.
_diag = [(b, rc, rc // 4) for b in range(B) for rc in range(32)]        # 64
_upper = [(b, rc, cc) for b in range(B) for rc in range(32)
          for cc in range(rc // 4 + 1, 8)]                              # 224
_diag_quad = [tuple(_diag[i:i + 4]) for i in range(0, 64, 4)]           # 16
_upper_quad = [tuple(_upper[i:i + 4]) for i in range(0, 224, 4)]        # 56
CLASH_QUAD_TABLE = [
    _diag_quad[2 * c:2 * c + 2] + _upper_quad[7 * c:7 * c + 7]
    for c in range(NCORES)
]                                                                       # 9/core
CQ = 9
# count-engine split per quad-tile index: 5 on ACT, 4 on DVE
ACT_IDX = {0, 2, 4, 6, 8}
_d = _a = 0
CPOS = {}
for _t in range(CQ):
    if _t in ACT_IDX:
        CPOS[_t] = ("a", _a); _a += 1
    else:
        CPOS[_t] = ("d", _d); _d += 1
N_ACT, N_DVE = _a, _d                                                   # 5, 4

# Physics compaction
PPAD = 384
PHYS_TILES = [(b, prc) for b in range(B) for prc in range(PPAD // 128)]  # 6

# out columns
OC_FAPE = 0                        # 4 cols (DVE accums)
OC_CD = OC_FAPE + len(FAPE_TILES)  # 4 cols (DVE counts)
OC_PH = OC_CD + N_DVE              # 1 col  (DVE)
OC_CA = OC_PH + 1                  # 5 cols (ACT sign sums)
OC_W = 16

QW = B * A                         # 8192  q cols
FQW = QW + B * RS                  # q | msym packed
CWW = CQ * 4 * 128                 # 4608  cwt cols
CMW = CQ * 4 * 512                 # 18432 cmt cols
CWM = CWW + CMW                    # cwt | cmt packed


def _build_nc():
    nc = bacc.Bacc("TRN2", target_bir_lowering=False, debug=False,
                   num_devices=NCORES)
    d_fq = nc.dram_tensor("fq", [28, FQW], F32R, kind="ExternalInput")
    d_cw = nc.dram_tensor("cw", [6, CWM], F32R, kind="ExternalInput")
    d_pp = nc.dram_tensor("pp", [5, 128 + PPAD], F32R, kind="ExternalInput")
    d_pmask = nc.dram_tensor("pmask", [128, PPAD], BF16, kind="ExternalInput")
    d_out = nc.dram_tensor("out", [128, OC_W], F32, kind="ExternalOutput")

    with TileContext(nc) as tc:
        with (
            tc.tile_pool(name="inp", bufs=1) as inp,
            tc.tile_pool(name="mps", bufs=2, space="PSUM") as mps,
            tc.tile_pool(name="scr", bufs=4) as scr,
            tc.tile_pool(name="accs", bufs=1) as accs,
        ):
            sb_fq = inp.tile([28, FQW], F32R, tag="fq")
            nc.sync.dma_start(sb_fq[:], d_fq[:])
            sb_cw = inp.tile([6, CWM], F32R, tag="cw")
            nc.sync.dma_start(sb_cw[:], d_cw[:])
            sb_pp = inp.tile([5, 128 + PPAD], F32R, tag="pp")
            nc.sync.dma_start(sb_pp[:], d_pp[:])
            sb_pmask = inp.tile([128, PPAD], BF16, tag="pmask")
            nc.sync.dma_start(sb_pmask[:], d_pmask[:])

            # DVE-written accumulators live in one tile (= DMA-ready slice);
            # ACT-written sign sums in their own tile (no cross-engine WAW).
            acc_dve = accs.tile([128, OC_CA], F32, tag="acc_dve")
            acc_ca = accs.tile([128, N_ACT], F32, tag="acc_ca")
            bias_c = accs.tile([128, 1], F32, tag="bias_c")
            nc.vector.memset(bias_c[:], SQRT_BIAS)

            def emit_fape(ti):
                b, h = FAPE_TILES[ti]
                ps = mps.tile([128, 2048], F32, tag="mp")
                for s in range(4):
                    a0 = b * A + h * 2048 + s * 512
                    nc.tensor.matmul(
                        ps[:, s * 512:(s + 1) * 512],
                        sb_fq[:, QW + b * RS: QW + (b + 1) * RS],
                        sb_fq[:, a0:a0 + 512],
                        start=True, stop=True)
                w = scr.tile([128, 2048], BF16, tag="fw")
                nc.scalar.activation(w[:], ps[:], ACTF.Sqrt, bias=bias_c[:])
                junk = scr.tile([128, 2048], BF16, tag="fj")
                nc.vector.tensor_scalar(
                    junk[:], w[:], CLAMP_DIST, None, ALU.min, ALU.add,
                    accum_out=acc_dve[:, OC_FAPE + ti:OC_FAPE + ti + 1])

            def emit_clash(t):
                ps = mps.tile([128, 2048], F32, tag="mp")
                for s in range(4):
                    t2 = 4 * t + s
                    nc.tensor.matmul(
                        ps[:, s * 512:(s + 1) * 512],
                        sb_cw[:, t2 * 128:(t2 + 1) * 128],
                        sb_cw[:, CWW + t2 * 512: CWW + (t2 + 1) * 512],
                        start=True, stop=True)
                s_ = scr.tile([128, 2048], BF16, tag="cs")
                kind, pos = CPOS[t]
                if kind == "d":
                    nc.vector.tensor_scalar(
                        s_[:], ps[:], 0.0, None, ALU.is_lt, ALU.add,
                        accum_out=acc_dve[:, OC_CD + pos:OC_CD + pos + 1])
                else:
                    nc.scalar.activation(
                        s_[:], ps[:], ACTF.Sign,
                        accum_out=acc_ca[:, pos:pos + 1])

            # interleave FAPE among clash tiles for engine overlap
            order = []
            fi, ci = 0, 0
            while fi < len(FAPE_TILES) or ci < CQ:
                if ci < CQ:
                    order.append(("c", ci)); ci += 1
                if fi < len(FAPE_TILES):
                    order.append(("f", fi)); fi += 1
                if ci < CQ:
                    order.append(("c", ci)); ci += 1
            for kind, ix in order:
                (emit_fape if kind == "f" else emit_clash)(ix)

            # ---- Physics ---- (PSUM slot from the shared pool)
            ps = mps.tile([128, PPAD], F32, tag="mp")
            nc.tensor.matmul(ps[:], sb_pp[:, :128], sb_pp[:, 128:],
                             start=True, stop=True)
            pv = scr.tile([128, PPAD], BF16, tag="pv")
            nc.vector.tensor_scalar(pv[:], ps[:], 1e-12, None, ALU.max)
            pd = scr.tile([128, PPAD], BF16, tag="pd")
            nc.scalar.activation(pd[:], pv[:], ACTF.Sqrt)
            p1 = scr.tile([128, PPAD], BF16, tag="p1")
            nc.vector.tensor_scalar(p1[:], pd[:], 1.33 + 0.2, 0.0,
                                    ALU.subtract, ALU.max)
            p2 = scr.tile([128, PPAD], BF16, tag="p2")
            nc.vector.tensor_scalar(p2[:], pd[:], 1.33 - 0.2, 0.0,
                                    ALU.subtract, ALU.min)
            pen = scr.tile([128, PPAD], BF16, tag="pen")
            nc.vector.tensor_sub(pen[:], p1[:], p2[:])
            pmm = scr.tile([128, PPAD], BF16, tag="pmm")
            nc.vector.tensor_mul(pmm[:], pen[:], sb_pmask[:])
            pj = scr.tile([128, PPAD], BF16, tag="pj")
            nc.vector.tensor_scalar(pj[:], pmm[:], 0.0, None, ALU.add, ALU.add,
                                    accum_out=acc_dve[:, OC_PH:OC_PH + 1])

            # merge ACT accumulators behind the DVE block, one DMA out
            out_sb = scr.tile([128, OC_W], F32, tag="out_sb")
            nc.vector.tensor_copy(out_sb[:, 0:OC_CA], acc_dve[:])
            nc.vector.tensor_copy(out_sb[:, OC_CA:OC_CA + N_ACT], acc_ca[:])
            nc.vector.memset(out_sb[:, OC_CA + N_ACT:], 0.0)
            nc.sync.dma_start(d_out[:], out_sb[:])
    nc.compile()
    return nc


_NC_CACHE = []


def _get_nc():
    if not _NC_CACHE:
        _NC_CACHE.append(_build_nc())
    return _NC_CACHE[0]


def _pack_inputs(inputs):
    """Host-side packing: returns (in_maps, host) for the device program."""
    rp = np.asarray(inputs["rots_pred"], dtype=np.float64)
    tp = np.asarray(inputs["trans_pred"], dtype=np.float64)
    xp = np.asarray(inputs["coords_pred"], dtype=np.float64)
    rt = np.asarray(inputs["rots_true"], dtype=np.float64)
    tt = np.asarray(inputs["trans_true"], dtype=np.float64)
    xt = np.asarray(inputs["coords_true"], dtype=np.float64)
    at = np.asarray(inputs["atom_types"])
    vr = np.asarray(inputs["vdw_radii"], dtype=np.float64)
    rm = np.asarray(inputs["res_mask"], dtype=np.float64)
    am = np.asarray(inputs["mask"], dtype=np.float64)

    # ---- FAPE msym / q ----
    c = (np.einsum("brji,brj->bri", rp, tp)
         - np.einsum("brji,brj->bri", rt, tt))                    # [B,R,3]
    G = np.concatenate([np.swapaxes(rp, -1, -2), -np.swapaxes(rt, -1, -2),
                        -c[..., None]], axis=-1)                  # [B,R,3,7]
    M = np.einsum("brki,brkj->brij", G, G)                        # [B,R,7,7]
    iu, ju = np.triu_indices(7)
    mult = np.where(iu == ju, 1.0, 2.0)
    msym = (M[:, :, iu, ju] * mult)                               # [B,R,28]
    x7 = np.concatenate([xp, xt, np.ones((B, A, 1))], axis=-1)    # [B,A,7]
    q = x7[:, :, iu] * x7[:, :, ju]                               # [B,A,28]

    # atom-mask handling: uniform per batch -> fold on host; 0/1 -> zero q
    m0 = np.empty(B)
    mask_corr = np.zeros(B)
    for b in range(B):
        vals = am[b]
        if np.all(vals == vals[0]):
            m0[b] = vals[0]
        elif np.all((vals == 0.0) | (vals == 1.0)):
            q[b, vals == 0.0, :] = 0.0
            m0[b] = 1.0
            mask_corr[b] = float((vals == 0.0).sum()) * np.sqrt(SQRT_BIAS)
        else:
            raise ValueError("unsupported non-{0,1} non-uniform atom mask")

    q_t = np.ascontiguousarray(
        q.transpose(2, 0, 1).reshape(28, B * A)).astype(np.float32)

    # ---- Clash weights/moving ----
    radii = vr[at]                                                # [B,A]
    nx = (xp * xp).sum(-1)                                        # [B,A]
    w6 = np.stack([-2 * xp[..., 0], -2 * xp[..., 1], -2 * xp[..., 2],
                   nx - radii ** 2, np.ones((B, A)), -2 * radii],
                  axis=1)                                         # [B,6,A]
    m6 = np.stack([xp[..., 0], xp[..., 1], xp[..., 2],
                   np.ones((B, A)), nx - radii ** 2, radii],
                  axis=1)                                         # [B,6,A]

    # ---- Physics compaction ----
    pw_all, pm_all, pmask_all, npairs = [], [], [], np.zeros(B)
    for b in range(B):
        ci = np.where(at[b] == C_IDX)[0]
        ni = np.where(at[b] == N_IDX)[0]
        nC, nN = len(ci), len(ni)
        assert nC <= PPAD and nN <= PPAD, (nC, nN)
        npairs[b] = max(nC * nN, 1.0)
        xc = np.zeros((PPAD, 3)); xc[:nC] = xp[b, ci]
        xn = np.zeros((PPAD, 3)); xn[:nN] = xp[b, ni]
        vc = np.zeros(PPAD); vc[:nC] = 1.0
        vn = np.zeros(PPAD); vn[:nN] = 1.0
        ncx = (xc * xc).sum(-1)
        nny = (xn * xn).sum(-1)
        pw_all.append(np.stack([-2 * xc[:, 0], -2 * xc[:, 1], -2 * xc[:, 2],
                                ncx, vc]))                        # [5,PPAD]
        pm_all.append(np.stack([xn[:, 0], xn[:, 1], xn[:, 2], vn, nny]))
        pmask_all.append(np.outer(vc, vn))                        # [PPAD,PPAD]

    try:
        import ml_dtypes
        bf16 = ml_dtypes.bfloat16
    except ImportError:  # pragma: no cover
        import jax.numpy as jnp
        bf16 = jnp.bfloat16

    # ---- per-core in_maps ----
    in_maps = []
    for cix in range(NCORES):
        msym_t = np.ascontiguousarray(
            msym[:, cix * RS:(cix + 1) * RS, :].transpose(2, 0, 1)
            .reshape(28, B * RS))
        fq = np.concatenate([q_t, msym_t.astype(np.float32)],
                            axis=1).astype(np.float32)
        subs = [st for quad in CLASH_QUAD_TABLE[cix] for st in quad]  # 36
        cwt = np.concatenate(
            [w6[b][:, rc * 128:(rc + 1) * 128] for (b, rc, cc) in subs],
            axis=1)
        cmt = np.concatenate(
            [m6[b][:, cc * 512:(cc + 1) * 512] for (b, rc, cc) in subs],
            axis=1)
        cw = np.concatenate([cwt, cmt], axis=1).astype(np.float32)
        if cix < len(PHYS_TILES):
            b, prc = PHYS_TILES[cix]
            pw = pw_all[b][:, prc * 128:(prc + 1) * 128]
            pm = pm_all[b]
            pmask = pmask_all[b][prc * 128:(prc + 1) * 128, :]
        else:
            pw = np.zeros((5, 128)); pm = np.zeros((5, PPAD))
            pmask = np.zeros((128, PPAD))
        pp = np.concatenate([pw, pm], axis=1).astype(np.float32)
        in_maps.append({
            "fq": fq,
            "cw": cw,
            "pp": pp,
            "pmask": pmask.astype(bf16),
        })

    host = dict(rm=rm, am=am, m0=m0, mask_corr=mask_corr, npairs=npairs)
    return in_maps, host


def _combine(outs, host):
    rm, am, m0 = host["rm"], host["am"], host["m0"]
    mask_corr, npairs = host["mask_corr"], host["npairs"]
    nf = A // 2048                        # fape tiles per batch

    S_err = 0.0
    for cix in range(NCORES):
        o = outs[cix].astype(np.float64)
        for b in range(B):
            rowsum = o[:, OC_FAPE + b * nf:OC_FAPE + (b + 1) * nf].sum(axis=1)
            rowsum = rowsum - mask_corr[b]
            S_err += float((rowsum * rm[b, cix * RS:(cix + 1) * RS]).sum()) * m0[b]
    fape = S_err / (am.sum() * rm.sum() + EPS)

    counts = np.zeros(B)
    for cix in range(NCORES):
        o = outs[cix].astype(np.float64)
        for t, quad in enumerate(CLASH_QUAD_TABLE[cix]):
            (b, rc, cc) = quad[0]
            wgt = 0.5 if cc == rc // 4 else 1.0
            kind, pos = CPOS[t]
            if kind == "d":
                cnt = o[:, OC_CD + pos].sum()
            else:
                cnt = (2048 * 128 - o[:, OC_CA + pos].sum()) / 2.0
            counts[b] += wgt * cnt
    clash = float(np.mean(counts / A))

    ph = np.zeros(B)
    for k, (b, prc) in enumerate(PHYS_TILES):
        ph[b] += outs[k][:, OC_PH].astype(np.float64).sum()
    physics = float(np.mean(ph / npairs))

    total = fape + CLASH_W * clash + PHYS_W * physics
    return np.float32(total), (fape, clash, physics)


def kernel(**inputs):
    nc = _get_nc()
    in_maps, host = _pack_inputs(inputs)
    res = run_bass_kernel_spmd(nc, in_maps, core_ids=list(range(NCORES)))
    outs = [res.results[c]["out"] for c in range(NCORES)]
    total, _ = _combine(outs, host)
    return np.asarray(total, dtype=np.float32)


# revision 9
# speedup vs baseline: 1.2049x; 1.2049x over previous
"""Trainium2 Bass kernel for nn_FAPELoss (B=2, R=1024, A=4096) on 8 NeuronCores.

Decomposition (all heavy compute on device):
  FAPE:  err^2[b,r,a] = <msym[b,r], q[b,a]> (28-dim symmetric-packed quadratic
         form of x7 = [coords_pred, coords_true, 1]) -> one K=28 fp32r matmul
         per [128 frames x 1024 atoms] tile.  Frames sharded across cores.
         Per tile: ACT sqrt(err^2 + BIAS) from PSUM (BIAS makes the argument
         provably positive under fp32r rounding; the systematic error is
         ~BIAS/(2*err) per point, far below tolerance), then DVE
         min(.,10)+row-accumulate in bf16 2x mode.
  Clash: u = d^2 - (r_i+r_j)^2 straight out of a K=6 fp32r matmul
         (weights [-2x; |x|^2-r^2; 1; -2r], moving [y; 1; |y|^2-r^2; r]);
         clash pair <=> u < 0 (the reference's d>EPS arm is always true due
         to its 1e-12 floor).  Upper-block-triangle of the symmetric AxA
         matrix only: count = S_upper + S_diagblocks/2.  Pairs of [128x512]
         blocks share one [128x1024] PSUM tile; the count is one
         DVE tensor_scalar(is_lt,accum) or ACT Sign(accum) per dual tile,
         split across both engines for balance.
  Physics: C/N atoms compacted on host (~220 of 4096 each) into a padded
         [384x384] problem; penalty relu(|d-1.33|-0.2), masked, accumulated.
         One [128x384] tile per core (6 used, 2 dummy).
Final tiny reductions (128-partition sums, res_mask weighting, denominators)
happen on host from a [128, 32] per-core accumulator tensor.
"""
import numpy as np

import concourse.bacc as bacc
import concourse.mybir as mybir
from concourse.tile import TileContext
from concourse.bass_utils import run_bass_kernel_spmd

F32 = mybir.dt.float32
F32R = mybir.dt.float32r
BF16 = mybir.dt.bfloat16
ALU = mybir.AluOpType
ACTF = mybir.ActivationFunctionType

# Problem constants (fixed by the module being modelled).
B, R, A = 2, 1024, 4096
NCORES = 8
RS = R // NCORES               # frames per core per batch = 128
CLAMP_DIST = 10.0
EPS = 1e-8
SQRT_BIAS = 0.02               # positivity guard for sqrt under fp32r rounding
C_IDX, N_IDX = 0, 1
CLASH_W, PHYS_W = 0.05, 0.3

# FAPE tiles: [128 frames x 1024 atoms]; 4 per batch -> 8 per core.
FAPE_TILES = [(b, h) for b in range(B) for h in range(A // 1024)]  # 8

# Clash dual tiles: the AxA (per batch) matrix in [128 x 512] blocks,
# upper block-triangle; two same-kind blocks share one [128 x 1024] PSUM
# tile so one count instruction covers both.
_diag = [(b, rc, rc // 4) for b in range(B) for rc in range(32)]        # 64
_upper = [(b, rc, cc) for b in range(B) for rc in range(32)
          for cc in range(rc // 4 + 1, 8)]                              # 224
_diag_dual = [(_diag[i], _diag[i + 1]) for i in range(0, 64, 2)]        # 32
_upper_dual = [(_upper[i], _upper[i + 1]) for i in range(0, 224, 2)]    # 112
CLASH_DUAL_TABLE = [
    _diag_dual[4 * c:4 * c + 4] + _upper_dual[14 * c:14 * c + 14]
    for c in range(NCORES)
]                                                                       # 18/core
CD = 18
# count-engine split per dual-tile index: 7 on ACT, 11 on DVE
ACT_IDX = {2, 5, 8, 10, 12, 14, 16}
_d = _a = 0
CPOS = {}
for _t in range(CD):
    if _t in ACT_IDX:
        CPOS[_t] = ("a", _a); _a += 1
    else:
        CPOS[_t] = ("d", _d); _d += 1
N_ACT, N_DVE = _a, _d                                                   # 8, 10

# Physics compaction
PPAD = 384
PHYS_TILES = [(b, prc) for b in range(B) for prc in range(PPAD // 128)]  # 6

# out columns
OC_FAPE = 0                       # 8 cols
OC_CA = OC_FAPE + len(FAPE_TILES)  # 8 cols (sign sums)
OC_CD = OC_CA + N_ACT             # 10 cols (counts)
OC_PH = OC_CD + N_DVE             # 1 col
OC_W = 32

QW = B * A                        # 8192  q cols
FQW = QW + B * RS                 # q | msym packed
CWW = CD * 2 * 128                # 4608  cwt cols
CMW = CD * 2 * 512                # 18432 cmt cols
CWM = CWW + CMW                   # cwt | cmt packed


def _build_nc():
    nc = bacc.Bacc("TRN2", target_bir_lowering=False, debug=False,
                   num_devices=NCORES)
    d_fq = nc.dram_tensor("fq", [28, FQW], F32R, kind="ExternalInput")
    d_cw = nc.dram_tensor("cw", [6, CWM], F32R, kind="ExternalInput")
    d_pp = nc.dram_tensor("pp", [5, 128 + PPAD], F32R, kind="ExternalInput")
    d_pmask = nc.dram_tensor("pmask", [128, PPAD], BF16, kind="ExternalInput")
    d_out = nc.dram_tensor("out", [128, OC_W], F32, kind="ExternalOutput")

    with TileContext(nc) as tc:
        with (
            tc.tile_pool(name="inp", bufs=1) as inp,
            tc.tile_pool(name="mps", bufs=3, space="PSUM") as mps,
            tc.tile_pool(name="pps", bufs=1, space="PSUM") as pps,
            tc.tile_pool(name="scr", bufs=4) as scr,
            tc.tile_pool(name="accs", bufs=1) as accs,
        ):
            # physics inputs first (tiny): its serial op chain overlaps the
            # big fq/cw transfers
            sb_pp = inp.tile([5, 128 + PPAD], F32R, tag="pp")
            nc.sync.dma_start(sb_pp[:], d_pp[:])
            sb_pmask = inp.tile([128, PPAD], BF16, tag="pmask")
            nc.sync.dma_start(sb_pmask[:], d_pmask[:])
            sb_fq = inp.tile([28, FQW], F32R, tag="fq")
            nc.sync.dma_start(sb_fq[:], d_fq[:])
            sb_cw = inp.tile([6, CWM], F32R, tag="cw")
            nc.sync.dma_start(sb_cw[:], d_cw[:])

            acc_f = accs.tile([128, len(FAPE_TILES)], F32, tag="acc_f")
            acc_ca = accs.tile([128, N_ACT], F32, tag="acc_ca")
            acc_cd = accs.tile([128, N_DVE], F32, tag="acc_cd")
            acc_ph = accs.tile([128, 1], F32, tag="acc_ph")
            bias_c = accs.tile([128, 1], F32, tag="bias_c")
            nc.vector.memset(bias_c[:], SQRT_BIAS)

            def emit_fape(ti):
                b, h = FAPE_TILES[ti]
                ps = mps.tile([128, 1024], F32, tag="mp")
                for s in range(2):
                    a0 = b * A + h * 1024 + s * 512
                    nc.tensor.matmul(
                        ps[:, s * 512:(s + 1) * 512],
                        sb_fq[:, QW + b * RS: QW + (b + 1) * RS],
                        sb_fq[:, a0:a0 + 512],
                        start=True, stop=True)
                w = scr.tile([128, 1024], BF16, tag="fw")
                nc.scalar.activation(w[:], ps[:], ACTF.Sqrt, bias=bias_c[:])
                junk = scr.tile([128, 1024], BF16, tag="fj")
                nc.vector.tensor_scalar(
                    junk[:], w[:], CLAMP_DIST, None, ALU.min, ALU.add,
                    accum_out=acc_f[:, ti:ti + 1])

            def emit_clash(t):
                ps = mps.tile([128, 1024], F32, tag="mp")
                for s in range(2):
                    t2 = 2 * t + s
                    nc.tensor.matmul(
                        ps[:, s * 512:(s + 1) * 512],
                        sb_cw[:, t2 * 128:(t2 + 1) * 128],
                        sb_cw[:, CWW + t2 * 512: CWW + (t2 + 1) * 512],
                        start=True, stop=True)
                s_ = scr.tile([128, 1024], BF16, tag="cs")
                kind, pos = CPOS[t]
                if kind == "d":
                    nc.vector.tensor_scalar(
                        s_[:], ps[:], 0.0, None, ALU.is_lt, ALU.add,
                        accum_out=acc_cd[:, pos:pos + 1])
                else:
                    nc.scalar.activation(
                        s_[:], ps[:], ACTF.Sign,
                        accum_out=acc_ca[:, pos:pos + 1])

            # ---- Physics first: overlaps the fq/cw DMA wait ----
            ps = pps.tile([128, PPAD], F32, tag="pp2")
            nc.tensor.matmul(ps[:], sb_pp[:, :128], sb_pp[:, 128:],
                             start=True, stop=True)
            pv = scr.tile([128, PPAD], BF16, tag="pv")
            nc.vector.tensor_scalar(pv[:], ps[:], 1e-12, None, ALU.max)
            pd = scr.tile([128, PPAD], BF16, tag="pd")
            nc.scalar.activation(pd[:], pv[:], ACTF.Sqrt)
            p1 = scr.tile([128, PPAD], BF16, tag="p1")
            nc.vector.tensor_scalar(p1[:], pd[:], 1.33 + 0.2, 0.0,
                                    ALU.subtract, ALU.max)
            p2 = scr.tile([128, PPAD], BF16, tag="p2")
            nc.vector.tensor_scalar(p2[:], pd[:], 1.33 - 0.2, 0.0,
                                    ALU.subtract, ALU.min)
            pen = scr.tile([128, PPAD], BF16, tag="pen")
            nc.vector.tensor_sub(pen[:], p1[:], p2[:])
            pmm = scr.tile([128, PPAD], BF16, tag="pmm")
            nc.vector.tensor_mul(pmm[:], pen[:], sb_pmask[:])
            pj = scr.tile([128, PPAD], BF16, tag="pj")
            nc.vector.tensor_scalar(pj[:], pmm[:], 0.0, None, ALU.add, ALU.add,
                                    accum_out=acc_ph[:, 0:1])

            # interleave FAPE among clash tiles for engine overlap
            order = []
            fi, ci = 0, 0
            while fi < len(FAPE_TILES) or ci < CD:
                if fi < len(FAPE_TILES):
                    order.append(("f", fi)); fi += 1
                if ci < CD:
                    order.append(("c", ci)); ci += 1
                if ci < CD and ci % 2 == 0:
                    order.append(("c", ci)); ci += 1
            for kind, ix in order:
                (emit_fape if kind == "f" else emit_clash)(ix)

            # merge accumulators (all on DVE) then one DMA out
            out_sb = scr.tile([128, OC_W], F32, tag="out_sb")
            nc.vector.tensor_copy(out_sb[:, OC_FAPE:OC_FAPE + len(FAPE_TILES)],
                                  acc_f[:])
            nc.vector.tensor_copy(out_sb[:, OC_CA:OC_CA + N_ACT], acc_ca[:])
            nc.vector.tensor_copy(out_sb[:, OC_CD:OC_CD + N_DVE], acc_cd[:])
            nc.vector.tensor_copy(out_sb[:, OC_PH:OC_PH + 1], acc_ph[:])
            nc.vector.memset(out_sb[:, OC_PH + 1:], 0.0)
            nc.sync.dma_start(d_out[:], out_sb[:])
    nc.compile()
    return nc


_NC_CACHE = []


def _get_nc():
    if not _NC_CACHE:
        _NC_CACHE.append(_build_nc())
    return _NC_CACHE[0]


def _pack_inputs(inputs):
    """Host-side packing: returns (in_maps, host) for the device program."""
    rp = np.asarray(inputs["rots_pred"], dtype=np.float64)
    tp = np.asarray(inputs["trans_pred"], dtype=np.float64)
    xp = np.asarray(inputs["coords_pred"], dtype=np.float64)
    rt = np.asarray(inputs["rots_true"], dtype=np.float64)
    tt = np.asarray(inputs["trans_true"], dtype=np.float64)
    xt = np.asarray(inputs["coords_true"], dtype=np.float64)
    at = np.asarray(inputs["atom_types"])
    vr = np.asarray(inputs["vdw_radii"], dtype=np.float64)
    rm = np.asarray(inputs["res_mask"], dtype=np.float64)
    am = np.asarray(inputs["mask"], dtype=np.float64)

    # ---- FAPE msym / q ----
    c = (np.einsum("brji,brj->bri", rp, tp)
         - np.einsum("brji,brj->bri", rt, tt))                    # [B,R,3]
    G = np.concatenate([np.swapaxes(rp, -1, -2), -np.swapaxes(rt, -1, -2),
                        -c[..., None]], axis=-1)                  # [B,R,3,7]
    M = np.einsum("brki,brkj->brij", G, G)                        # [B,R,7,7]
    iu, ju = np.triu_indices(7)
    mult = np.where(iu == ju, 1.0, 2.0)
    msym = (M[:, :, iu, ju] * mult)                               # [B,R,28]
    x7 = np.concatenate([xp, xt, np.ones((B, A, 1))], axis=-1)    # [B,A,7]
    q = x7[:, :, iu] * x7[:, :, ju]                               # [B,A,28]

    # atom-mask handling: uniform per batch -> fold on host; 0/1 -> zero q
    m0 = np.empty(B)
    mask_corr = np.zeros(B)
    for b in range(B):
        vals = am[b]
        if np.all(vals == vals[0]):
            m0[b] = vals[0]
        elif np.all((vals == 0.0) | (vals == 1.0)):
            q[b, vals == 0.0, :] = 0.0
            m0[b] = 1.0
            mask_corr[b] = float((vals == 0.0).sum()) * np.sqrt(SQRT_BIAS)
        else:
            raise ValueError("unsupported non-{0,1} non-uniform atom mask")

    q_t = np.ascontiguousarray(
        q.transpose(2, 0, 1).reshape(28, B * A)).astype(np.float32)

    # ---- Clash weights/moving ----
    radii = vr[at]                                                # [B,A]
    nx = (xp * xp).sum(-1)                                        # [B,A]
    w6 = np.stack([-2 * xp[..., 0], -2 * xp[..., 1], -2 * xp[..., 2],
                   nx - radii ** 2, np.ones((B, A)), -2 * radii],
                  axis=1)                                         # [B,6,A]
    m6 = np.stack([xp[..., 0], xp[..., 1], xp[..., 2],
                   np.ones((B, A)), nx - radii ** 2, radii],
                  axis=1)                                         # [B,6,A]

    # ---- Physics compaction ----
    pw_all, pm_all, pmask_all, npairs = [], [], [], np.zeros(B)
    for b in range(B):
        ci = np.where(at[b] == C_IDX)[0]
        ni = np.where(at[b] == N_IDX)[0]
        nC, nN = len(ci), len(ni)
        assert nC <= PPAD and nN <= PPAD, (nC, nN)
        npairs[b] = max(nC * nN, 1.0)
        xc = np.zeros((PPAD, 3)); xc[:nC] = xp[b, ci]
        xn = np.zeros((PPAD, 3)); xn[:nN] = xp[b, ni]
        vc = np.zeros(PPAD); vc[:nC] = 1.0
        vn = np.zeros(PPAD); vn[:nN] = 1.0
        ncx = (xc * xc).sum(-1)
        nny = (xn * xn).sum(-1)
        pw_all.append(np.stack([-2 * xc[:, 0], -2 * xc[:, 1], -2 * xc[:, 2],
                                ncx, vc]))                        # [5,PPAD]
        pm_all.append(np.stack([xn[:, 0], xn[:, 1], xn[:, 2], vn, nny]))
        pmask_all.append(np.outer(vc, vn))                        # [PPAD,PPAD]

    try:
        import ml_dtypes
        bf16 = ml_dtypes.bfloat16
    except ImportError:  # pragma: no cover
        import jax.numpy as jnp
        bf16 = jnp.bfloat16

    # ---- per-core in_maps ----
    in_maps = []
    for cix in range(NCORES):
        msym_t = np.ascontiguousarray(
            msym[:, cix * RS:(cix + 1) * RS, :].transpose(2, 0, 1)
            .reshape(28, B * RS))
        fq = np.concatenate([q_t, msym_t.astype(np.float32)],
                            axis=1).astype(np.float32)
        subs = [st for dual in CLASH_DUAL_TABLE[cix] for st in dual]  # 36
        cwt = np.concatenate(
            [w6[b][:, rc * 128:(rc + 1) * 128] for (b, rc, cc) in subs],
            axis=1)
        cmt = np.concatenate(
            [m6[b][:, cc * 512:(cc + 1) * 512] for (b, rc, cc) in subs],
            axis=1)
        cw = np.concatenate([cwt, cmt], axis=1).astype(np.float32)
        if cix < len(PHYS_TILES):
            b, prc = PHYS_TILES[cix]
            pw = pw_all[b][:, prc * 128:(prc + 1) * 128]
            pm = pm_all[b]
            pmask = pmask_all[b][prc * 128:(prc + 1) * 128, :]
        else:
            pw = np.zeros((5, 128)); pm = np.zeros((5, PPAD))
            pmask = np.zeros((128, PPAD))
        pp = np.concatenate([pw, pm], axis=1).astype(np.float32)
        in_maps.append({
            "fq": fq,
            "cw": cw,
            "pp": pp,
            "pmask": pmask.astype(bf16),
        })

    host = dict(rm=rm, am=am, m0=m0, mask_corr=mask_corr, npairs=npairs)
    return in_maps, host


def _combine(outs, host):
    rm, am, m0 = host["rm"], host["am"], host["m0"]
    mask_corr, npairs = host["mask_corr"], host["npairs"]
    nf = A // 1024                        # fape tiles per batch

    S_err = 0.0
    for cix in range(NCORES):
        o = outs[cix].astype(np.float64)
        for b in range(B):
            rowsum = o[:, OC_FAPE + b * nf:OC_FAPE + (b + 1) * nf].sum(axis=1)
            rowsum = rowsum - mask_corr[b]
            S_err += float((rowsum * rm[b, cix * RS:(cix + 1) * RS]).sum()) * m0[b]
    fape = S_err / (am.sum() * rm.sum() + EPS)

    counts = np.zeros(B)
    for cix in range(NCORES):
        o = outs[cix].astype(np.float64)
        for t, dual in enumerate(CLASH_DUAL_TABLE[cix]):
            (b, rc, cc), _ = dual
            wgt = 0.5 if cc == rc // 4 else 1.0
            kind, pos = CPOS[t]
            if kind == "d":
                cnt = o[:, OC_CD + pos].sum()
            else:
                cnt = (1024 * 128 - o[:, OC_CA + pos].sum()) / 2.0
            counts[b] += wgt * cnt
    clash = float(np.mean(counts / A))

    ph = np.zeros(B)
    for k, (b, prc) in enumerate(PHYS_TILES):
        ph[b] += outs[k][:, OC_PH].astype(np.float64).sum()
    physics = float(np.mean(ph / npairs))

    total = fape + CLASH_W * clash + PHYS_W * physics
    return np.float32(total), (fape, clash, physics)


def kernel(**inputs):
    nc = _get_nc()
    in_maps, host = _pack_inputs(inputs)
    res = run_bass_kernel_spmd(nc, in_maps, core_ids=list(range(NCORES)))
    outs = [res.results[c]["out"] for c in range(NCORES)]
    total, _ = _combine(outs, host)
    return np.asarray(total, dtype=np.float32)


# revision 10
# speedup vs baseline: 1.2733x; 1.0568x over previous
"""Trainium2 Bass kernel for nn_FAPELoss (B=2, R=1024, A=4096) on 8 NeuronCores.

Decomposition (all heavy compute on device):
  FAPE:  err^2[b,r,a] = <msym[b,r], q[b,a]> (28-dim symmetric-packed quadratic
         form of x7 = [coords_pred, coords_true, 1]) -> one K=28 fp32r matmul
         per [128 frames x 1024 atoms] tile.  Frames sharded across cores.
         Per tile: ACT sqrt(err^2 + BIAS) from PSUM (BIAS makes the argument
         provably positive under fp32r rounding; the systematic error is
         ~BIAS/(2*err) per point, far below tolerance), then DVE
         min(.,10)+row-accumulate in bf16 2x mode.
  Clash: u = d^2 - (r_i+r_j)^2 straight out of a K=6 fp32r matmul
         (weights [-2x; |x|^2-r^2; 1; -2r], moving [y; 1; |y|^2-r^2; r]);
         clash pair <=> u < 0 (the reference's d>EPS arm is always true due
         to its 1e-12 floor).  Upper-block-triangle of the symmetric AxA
         matrix only: count = S_upper + S_diagblocks/2.  Pairs of [128x512]
         blocks share one [128x1024] PSUM tile; the count is one
         DVE tensor_scalar(is_lt,accum) or ACT Sign(accum) per dual tile,
         split across both engines for balance.
  Physics: C/N atoms compacted on host (~220 of 4096 each) into a padded
         [384x384] problem; penalty relu(|d-1.33|-0.2), masked, accumulated.
         One [128x384] tile per core (6 used, 2 dummy).
Final tiny reductions (128-partition sums, res_mask weighting, denominators)
happen on host from a [128, 32] per-core accumulator tensor.
"""
import numpy as np

import concourse.bacc as bacc
import concourse.mybir as mybir
from concourse.tile import TileContext
from concourse.bass_utils import run_bass_kernel_spmd

F32 = mybir.dt.float32
F32R = mybir.dt.float32r
BF16 = mybir.dt.bfloat16
ALU = mybir.AluOpType
ACTF = mybir.ActivationFunctionType

# Problem constants (fixed by the module being modelled).
B, R, A = 2, 1024, 4096
NCORES = 8
RS = R // NCORES               # frames per core per batch = 128
CLAMP_DIST = 10.0
EPS = 1e-8
SQRT_BIAS = 0.02               # positivity guard for sqrt under fp32r rounding
C_IDX, N_IDX = 0, 1
CLASH_W, PHYS_W = 0.05, 0.3

# FAPE tiles: [128 frames x 1024 atoms]; 4 per batch -> 8 per core.
FAPE_TILES = [(b, h) for b in range(B) for h in range(A // 1024)]  # 8

# Clash dual tiles: the AxA (per batch) matrix in [128 x 512] blocks,
# upper block-triangle; two same-kind blocks share one [128 x 1024] PSUM
# tile so one count instruction covers both.
_diag = [(b, rc, rc // 4) for b in range(B) for rc in range(32)]        # 64
_upper = [(b, rc, cc) for b in range(B) for rc in range(32)
          for cc in range(rc // 4 + 1, 8)]                              # 224
_diag_dual = [(_diag[i], _diag[i + 1]) for i in range(0, 64, 2)]        # 32
_upper_dual = [(_upper[i], _upper[i + 1]) for i in range(0, 224, 2)]    # 112
CLASH_DUAL_TABLE = [
    _diag_dual[4 * c:4 * c + 4] + _upper_dual[14 * c:14 * c + 14]
    for c in range(NCORES)
]                                                                       # 18/core
CD = 18
# count-engine split per dual-tile index: 7 on ACT, 11 on DVE
ACT_IDX = {2, 5, 8, 10, 12, 14, 16}
_d = _a = 0
CPOS = {}
for _t in range(CD):
    if _t in ACT_IDX:
        CPOS[_t] = ("a", _a); _a += 1
    else:
        CPOS[_t] = ("d", _d); _d += 1
N_ACT, N_DVE = _a, _d                                                   # 8, 10

# Physics compaction
PPAD = 384
PHYS_TILES = [(b, prc) for b in range(B) for prc in range(PPAD // 128)]  # 6

# out columns
OC_FAPE = 0                       # 8 cols
OC_CA = OC_FAPE + len(FAPE_TILES)  # 8 cols (sign sums)
OC_CD = OC_CA + N_ACT             # 10 cols (counts)
OC_PH = OC_CD + N_DVE             # 1 col
OC_W = 32

QW = B * A                        # 8192  q cols
MW = B * RS                       # 256 msym cols
FQW = QW + MW                     # msym | q packed
CWW = CD * 2 * 128                # 4608  cwt cols
CMW = CD * 2 * 512                # 18432 cmt cols
CWM = CWW + CMW                   # cwt | cmt packed


def _build_nc():
    nc = bacc.Bacc("TRN2", target_bir_lowering=False, debug=False,
                   num_devices=NCORES)
    d_fq = nc.dram_tensor("fq", [28, FQW], F32R, kind="ExternalInput")
    d_cw = nc.dram_tensor("cw", [6, CWM], F32R, kind="ExternalInput")
    d_pp = nc.dram_tensor("pp", [5, 128 + PPAD], F32R, kind="ExternalInput")
    d_pmask = nc.dram_tensor("pmask", [128, PPAD], BF16, kind="ExternalInput")
    d_out = nc.dram_tensor("out", [128, OC_W], F32, kind="ExternalOutput")

    with TileContext(nc) as tc:
        with (
            tc.tile_pool(name="inp", bufs=1) as inp,
            tc.tile_pool(name="mps", bufs=3, space="PSUM") as mps,
            tc.tile_pool(name="pps", bufs=1, space="PSUM") as pps,
            tc.tile_pool(name="scr", bufs=4) as scr,
            tc.tile_pool(name="accs", bufs=1) as accs,
        ):
            # physics inputs first (tiny): its serial op chain overlaps the
            # big fq/cw transfers
            sb_pp = inp.tile([5, 128 + PPAD], F32R, tag="pp")
            nc.sync.dma_start(sb_pp[:], d_pp[:])
            sb_pmask = inp.tile([128, PPAD], BF16, tag="pmask")
            nc.sync.dma_start(sb_pmask[:], d_pmask[:])
            sb_fq = inp.tile([28, FQW], F32R, tag="fq")
            sb_cw = inp.tile([6, CWM], F32R, tag="cw")
            half_cm = CWW + CMW // 2
            half_fq = MW + QW // 2
            nc.sync.dma_start(sb_cw[:, :half_cm], d_cw[:, :half_cm])
            nc.sync.dma_start(sb_fq[:, :half_fq], d_fq[:, :half_fq])
            nc.sync.dma_start(sb_cw[:, half_cm:], d_cw[:, half_cm:])
            nc.sync.dma_start(sb_fq[:, half_fq:], d_fq[:, half_fq:])

            acc_f = accs.tile([128, len(FAPE_TILES)], F32, tag="acc_f")
            acc_ca = accs.tile([128, N_ACT], F32, tag="acc_ca")
            acc_cd = accs.tile([128, N_DVE], F32, tag="acc_cd")
            acc_ph = accs.tile([128, 1], F32, tag="acc_ph")
            bias_c = accs.tile([128, 1], F32, tag="bias_c")
            nc.vector.memset(bias_c[:], SQRT_BIAS)

            def emit_fape(ti):
                b, h = FAPE_TILES[ti]
                ps = mps.tile([128, 1024], F32, tag="mp")
                for s in range(2):
                    a0 = b * A + h * 1024 + s * 512
                    nc.tensor.matmul(
                        ps[:, s * 512:(s + 1) * 512],
                        sb_fq[:, b * RS:(b + 1) * RS],
                        sb_fq[:, MW + a0: MW + a0 + 512],
                        start=True, stop=True)
                w = scr.tile([128, 1024], BF16, tag="fw")
                nc.scalar.activation(w[:], ps[:], ACTF.Sqrt, bias=bias_c[:])
                junk = scr.tile([128, 1024], BF16, tag="fj")
                nc.vector.tensor_scalar(
                    junk[:], w[:], CLAMP_DIST, None, ALU.min, ALU.add,
                    accum_out=acc_f[:, ti:ti + 1])

            def emit_clash(t):
                ps = mps.tile([128, 1024], F32, tag="mp")
                for s in range(2):
                    t2 = 2 * t + s
                    nc.tensor.matmul(
                        ps[:, s * 512:(s + 1) * 512],
                        sb_cw[:, t2 * 128:(t2 + 1) * 128],
                        sb_cw[:, CWW + t2 * 512: CWW + (t2 + 1) * 512],
                        start=True, stop=True)
                s_ = scr.tile([128, 1024], BF16, tag="cs")
                kind, pos = CPOS[t]
                if kind == "d":
                    nc.vector.tensor_scalar(
                        s_[:], ps[:], 0.0, None, ALU.is_lt, ALU.add,
                        accum_out=acc_cd[:, pos:pos + 1])
                else:
                    nc.scalar.activation(
                        s_[:], ps[:], ACTF.Sign,
                        accum_out=acc_ca[:, pos:pos + 1])

            # ---- Physics first: overlaps the fq/cw DMA wait ----
            ps = pps.tile([128, PPAD], F32, tag="pp2")
            nc.tensor.matmul(ps[:], sb_pp[:, :128], sb_pp[:, 128:],
                             start=True, stop=True)
            pv = scr.tile([128, PPAD], BF16, tag="pv")
            nc.vector.tensor_scalar(pv[:], ps[:], 1e-12, None, ALU.max)
            pd = scr.tile([128, PPAD], BF16, tag="pd")
            nc.scalar.activation(pd[:], pv[:], ACTF.Sqrt)
            p1 = scr.tile([128, PPAD], BF16, tag="p1")
            nc.vector.tensor_scalar(p1[:], pd[:], 1.33 + 0.2, 0.0,
                                    ALU.subtract, ALU.max)
            p2 = scr.tile([128, PPAD], BF16, tag="p2")
            nc.vector.tensor_scalar(p2[:], pd[:], 1.33 - 0.2, 0.0,
                                    ALU.subtract, ALU.min)
            pen = scr.tile([128, PPAD], BF16, tag="pen")
            nc.vector.tensor_sub(pen[:], p1[:], p2[:])
            pmm = scr.tile([128, PPAD], BF16, tag="pmm")
            nc.vector.tensor_mul(pmm[:], pen[:], sb_pmask[:])
            pj = scr.tile([128, PPAD], BF16, tag="pj")
            nc.vector.tensor_scalar(pj[:], pmm[:], 0.0, None, ALU.add, ALU.add,
                                    accum_out=acc_ph[:, 0:1])

            # interleave FAPE among clash tiles for engine overlap
            order = []
            fi, ci = 0, 0
            while fi < len(FAPE_TILES) or ci < CD:
                if fi < len(FAPE_TILES):
                    order.append(("f", fi)); fi += 1
                if ci < CD:
                    order.append(("c", ci)); ci += 1
                if ci < CD and ci % 2 == 0:
                    order.append(("c", ci)); ci += 1
            for kind, ix in order:
                (emit_fape if kind == "f" else emit_clash)(ix)

            # merge accumulators (all on DVE) then one DMA out
            out_sb = scr.tile([128, OC_W], F32, tag="out_sb")
            nc.vector.tensor_copy(out_sb[:, OC_FAPE:OC_FAPE + len(FAPE_TILES)],
                                  acc_f[:])
            nc.vector.tensor_copy(out_sb[:, OC_CA:OC_CA + N_ACT], acc_ca[:])
            nc.vector.tensor_copy(out_sb[:, OC_CD:OC_CD + N_DVE], acc_cd[:])
            nc.vector.tensor_copy(out_sb[:, OC_PH:OC_PH + 1], acc_ph[:])
            nc.vector.memset(out_sb[:, OC_PH + 1:], 0.0)
            nc.sync.dma_start(d_out[:], out_sb[:])
    nc.compile()
    return nc


_NC_CACHE = []


def _get_nc():
    if not _NC_CACHE:
        _NC_CACHE.append(_build_nc())
    return _NC_CACHE[0]


def _pack_inputs(inputs):
    """Host-side packing: returns (in_maps, host) for the device program."""
    rp = np.asarray(inputs["rots_pred"], dtype=np.float64)
    tp = np.asarray(inputs["trans_pred"], dtype=np.float64)
    xp = np.asarray(inputs["coords_pred"], dtype=np.float64)
    rt = np.asarray(inputs["rots_true"], dtype=np.float64)
    tt = np.asarray(inputs["trans_true"], dtype=np.float64)
    xt = np.asarray(inputs["coords_true"], dtype=np.float64)
    at = np.asarray(inputs["atom_types"])
    vr = np.asarray(inputs["vdw_radii"], dtype=np.float64)
    rm = np.asarray(inputs["res_mask"], dtype=np.float64)
    am = np.asarray(inputs["mask"], dtype=np.float64)

    # ---- FAPE msym / q ----
    c = (np.einsum("brji,brj->bri", rp, tp)
         - np.einsum("brji,brj->bri", rt, tt))                    # [B,R,3]
    G = np.concatenate([np.swapaxes(rp, -1, -2), -np.swapaxes(rt, -1, -2),
                        -c[..., None]], axis=-1)                  # [B,R,3,7]
    M = np.einsum("brki,brkj->brij", G, G)                        # [B,R,7,7]
    iu, ju = np.triu_indices(7)
    mult = np.where(iu == ju, 1.0, 2.0)
    msym = (M[:, :, iu, ju] * mult)                               # [B,R,28]
    x7 = np.concatenate([xp, xt, np.ones((B, A, 1))], axis=-1)    # [B,A,7]
    q = x7[:, :, iu] * x7[:, :, ju]                               # [B,A,28]

    # atom-mask handling: uniform per batch -> fold on host; 0/1 -> zero q
    m0 = np.empty(B)
    mask_corr = np.zeros(B)
    for b in range(B):
        vals = am[b]
        if np.all(vals == vals[0]):
            m0[b] = vals[0]
        elif np.all((vals == 0.0) | (vals == 1.0)):
            q[b, vals == 0.0, :] = 0.0
            m0[b] = 1.0
            mask_corr[b] = float((vals == 0.0).sum()) * np.sqrt(SQRT_BIAS)
        else:
            raise ValueError("unsupported non-{0,1} non-uniform atom mask")

    q_t = np.ascontiguousarray(
        q.transpose(2, 0, 1).reshape(28, B * A)).astype(np.float32)

    # ---- Clash weights/moving ----
    radii = vr[at]                                                # [B,A]
    nx = (xp * xp).sum(-1)                                        # [B,A]
    w6 = np.stack([-2 * xp[..., 0], -2 * xp[..., 1], -2 * xp[..., 2],
                   nx - radii ** 2, np.ones((B, A)), -2 * radii],
                  axis=1)                                         # [B,6,A]
    m6 = np.stack([xp[..., 0], xp[..., 1], xp[..., 2],
                   np.ones((B, A)), nx - radii ** 2, radii],
                  axis=1)                                         # [B,6,A]

    # ---- Physics compaction ----
    pw_all, pm_all, pmask_all, npairs = [], [], [], np.zeros(B)
    for b in range(B):
        ci = np.where(at[b] == C_IDX)[0]
        ni = np.where(at[b] == N_IDX)[0]
        nC, nN = len(ci), len(ni)
        assert nC <= PPAD and nN <= PPAD, (nC, nN)
        npairs[b] = max(nC * nN, 1.0)
        xc = np.zeros((PPAD, 3)); xc[:nC] = xp[b, ci]
        xn = np.zeros((PPAD, 3)); xn[:nN] = xp[b, ni]
        vc = np.zeros(PPAD); vc[:nC] = 1.0
        vn = np.zeros(PPAD); vn[:nN] = 1.0
        ncx = (xc * xc).sum(-1)
        nny = (xn * xn).sum(-1)
        pw_all.append(np.stack([-2 * xc[:, 0], -2 * xc[:, 1], -2 * xc[:, 2],
                                ncx, vc]))                        # [5,PPAD]
        pm_all.append(np.stack([xn[:, 0], xn[:, 1], xn[:, 2], vn, nny]))
        pmask_all.append(np.outer(vc, vn))                        # [PPAD,PPAD]

    try:
        import ml_dtypes
        bf16 = ml_dtypes.bfloat16
    except ImportError:  # pragma: no cover
        import jax.numpy as jnp
        bf16 = jnp.bfloat16

    # ---- per-core in_maps ----
    in_maps = []
    for cix in range(NCORES):
        msym_t = np.ascontiguousarray(
            msym[:, cix * RS:(cix + 1) * RS, :].transpose(2, 0, 1)
            .reshape(28, B * RS))
        fq = np.concatenate([msym_t.astype(np.float32), q_t],
                            axis=1).astype(np.float32)
        subs = [st for dual in CLASH_DUAL_TABLE[cix] for st in dual]  # 36
        cwt = np.concatenate(
            [w6[b][:, rc * 128:(rc + 1) * 128] for (b, rc, cc) in subs],
            axis=1)
        cmt = np.concatenate(
            [m6[b][:, cc * 512:(cc + 1) * 512] for (b, rc, cc) in subs],
            axis=1)
        cw = np.concatenate([cwt, cmt], axis=1).astype(np.float32)
        if cix < len(PHYS_TILES):
            b, prc = PHYS_TILES[cix]
            pw = pw_all[b][:, prc * 128:(prc + 1) * 128]
            pm = pm_all[b]
            pmask = pmask_all[b][prc * 128:(prc + 1) * 128, :]
        else:
            pw = np.zeros((5, 128)); pm = np.zeros((5, PPAD))
            pmask = np.zeros((128, PPAD))
        pp = np.concatenate([pw, pm], axis=1).astype(np.float32)
        in_maps.append({
            "fq": fq,
            "cw": cw,
            "pp": pp,
            "pmask": pmask.astype(bf16),
        })

    host = dict(rm=rm, am=am, m0=m0, mask_corr=mask_corr, npairs=npairs)
    return in_maps, host


def _combine(outs, host):
    rm, am, m0 = host["rm"], host["am"], host["m0"]
    mask_corr, npairs = host["mask_corr"], host["npairs"]
    nf = A // 1024                        # fape tiles per batch

    S_err = 0.0
    for cix in range(NCORES):
        o = outs[cix].astype(np.float64)
        for b in range(B):
            rowsum = o[:, OC_FAPE + b * nf:OC_FAPE + (b + 1) * nf].sum(axis=1)
            rowsum = rowsum - mask_corr[b]
            S_err += float((rowsum * rm[b, cix * RS:(cix + 1) * RS]).sum()) * m0[b]
    fape = S_err / (am.sum() * rm.sum() + EPS)

    counts = np.zeros(B)
    for cix in range(NCORES):
        o = outs[cix].astype(np.float64)
        for t, dual in enumerate(CLASH_DUAL_TABLE[cix]):
            (b, rc, cc), _ = dual
            wgt = 0.5 if cc == rc // 4 else 1.0
            kind, pos = CPOS[t]
            if kind == "d":
                cnt = o[:, OC_CD + pos].sum()
            else:
                cnt = (1024 * 128 - o[:, OC_CA + pos].sum()) / 2.0
            counts[b] += wgt * cnt
    clash = float(np.mean(counts / A))

    ph = np.zeros(B)
    for k, (b, prc) in enumerate(PHYS_TILES):
        ph[b] += outs[k][:, OC_PH].astype(np.float64).sum()
    physics = float(np.mean(ph / npairs))

    total = fape + CLASH_W * clash + PHYS_W * physics
    return np.float32(total), (fape, clash, physics)


def kernel(**inputs):
    nc = _get_nc()
    in_maps, host = _pack_inputs(inputs)
    res = run_bass_kernel_spmd(nc, in_maps, core_ids=list(range(NCORES)))
    outs = [res.results[c]["out"] for c in range(NCORES)]
    total, _ = _combine(outs, host)
    return np.asarray(total, dtype=np.float32)
